# revision 2
# baseline (speedup 1.0000x reference)
"""CIN (Compressed Interaction Network) forward kernel for Trainium2.

Data-parallel over 8 NeuronCores: batch dim B=2048 is sharded 256/core,
conv weights are replicated. No cross-device communication.

Per-core layout: everything lives as (channels, n) where n = (b_local, d)
flattened to 8192 columns, processed in chunks of NC=512 columns.

Precision split (rel-err gate 2e-2; measured 1.1e-2 in simulation):
  - Layer 0 runs fp16: its hidden output feeds layers 1 and 2, so its
    error compounds. The folded x (x) x interaction products are
    precomputed on host (one rounding) and DMA'd as a single fp16 plane.
  - Layers 1/2 run fp8 e4m3 with DoubleRow perf mode (K=256 per
    instruction, 2x PE throughput). Weights are host-scaled by 64 into
    e4m3's normal range; the inverse scale rides the ScalarE activation.
Per K-pair-tile the rhs[(h,f), n] = hidden[h,n] * x0t[f,n] products are
built by DVE tensor_mul (fp16 hidden x fp8 T -> fp8 out); x0t rows are
broadcast to 128 partitions via one 2.6MB stride-0 DMA per chunk.
ReLU + bias(+1/64 scale) are fused into ScalarE activations; sum-over-D
runs as DVE segment-reduce into resident output tiles.
"""

import sys

if "/opt/trn_rl_repo" not in sys.path:
    sys.path.insert(0, "/opt/trn_rl_repo")

from contextlib import ExitStack

import ml_dtypes
import numpy as np

import concourse.bacc as bacc
import concourse.bass as bass
import concourse.mybir as mybir
import concourse.tile as tile
from concourse import bass_utils

# Problem shapes (hardcoded per contest rules)
B, F, D = 2048, 39, 32
O = 256          # conv output channels per layer
H = 128          # hidden channels fed to layers 1,2
NCORES = 8
B_LOC = B // NCORES          # 256 batches per core
N_LOC = B_LOC * D            # 8192 columns per core

NC = 512                     # columns per chunk
NB = NC // D                 # batches per chunk (16)
F_PAD = 40                   # f-slots padded even for DoubleRow pairing
NSL = 2                      # rhs build slices per layer
SG = F_PAD // NSL            # 20 f-slots per slice
PAIRS = SG // 2              # 10 DoubleRow K-pair-tiles per slice
# layer-0 symmetry folding: x0 (x) x0 is symmetric, keep pairs h <= f only
NPAIR = F * (F + 1) // 2     # 780
QG = (NPAIR + 127) // 128    # 7 K-tiles
Q = QG * 128                 # 896 padded rows
WS = 64.0                    # fp8 weight pre-scale (undone in activation)

F8 = mybir.dt.float8e4
F16 = mybir.dt.float16
F32 = mybir.dt.float32
AF = mybir.ActivationFunctionType
DR = mybir.MatmulPerfMode.DoubleRow
E4NP = ml_dtypes.float8_e4m3

TRACE = False                # set True from test harness to profile
_LAST_RESULTS = None         # BassKernelResults of last run (for test.py)


def build_module(b_loc=B_LOC, nc_cols=NC):
    """Build the Bass/Tile module for one core (shapes are per-core)."""
    n_loc = b_loc * D
    nchunk = n_loc // nc_cols
    nb = nc_cols // D
    assert n_loc % nc_cols == 0 and nc_cols % D == 0

    nc = bacc.Bacc("TRN2", target_bir_lowering=False, debug=False)

    xtc = nc.dram_tensor("xtc", (nchunk, F_PAD, nc_cols), F8, kind="ExternalInput").ap()
    rhs0c = nc.dram_tensor(
        "rhs0c", (nchunk, 128, QG * nc_cols), F16, kind="ExternalInput"
    ).ap()
    wt0 = nc.dram_tensor("wt0", (128, QG * O), F16, kind="ExternalInput").ap()
    wt1 = nc.dram_tensor("wt1", (128, F_PAD * O), F8, kind="ExternalInput").ap()
    wt2 = nc.dram_tensor("wt2", (128, F_PAD * O), F8, kind="ExternalInput").ap()
    biases = nc.dram_tensor("biases", (128, 8), F32, kind="ExternalInput").ap()
    out = nc.dram_tensor("out", (4, 128, b_loc), F32, kind="ExternalOutput").ap()

    with tile.TileContext(nc) as tc, ExitStack() as ctx:
        const = ctx.enter_context(tc.tile_pool(name="const", bufs=1))
        t_pool = ctx.enter_context(tc.tile_pool(name="tpool", bufs=3))
        rhs0_pool = ctx.enter_context(tc.tile_pool(name="r0pool", bufs=3))
        sl_pool = ctx.enter_context(tc.tile_pool(name="slpool", bufs=7))
        hid_pool = ctx.enter_context(tc.tile_pool(name="hidpool", bufs=4))
        d_pool = ctx.enter_context(tc.tile_pool(name="dpool", bufs=4))
        psum_pool = ctx.enter_context(tc.tile_pool(name="psum", bufs=8, space="PSUM"))

        # --- resident tensors ---
        wt0_sb = const.tile([128, QG, O], F16)
        wt1_sb = const.tile([128, F_PAD, O], F8)
        wt2_sb = const.tile([128, F_PAD, O], F8)
        bias_sb = const.tile([128, 8], F32)
        out_sb = [const.tile([128, b_loc], F32, name=f"osb{i}") for i in range(4)]

        # Preamble DMAs: only what chunk 0's layer 0 needs, in consumption
        # order on the SP ring. wt1/wt2 are emitted lazily (on the ACT
        # HWDGE ring) right before their first consumers so the startup
        # isn't HBM-bound on the weights.
        nc.sync.dma_start(bias_sb[:], biases)
        nc.sync.dma_start(wt0_sb[:], wt0.rearrange("p (g o) -> p g o", o=O))
        wt1_r = wt1.rearrange("p (f o) -> p f o", o=O)
        wt2_r = wt2.rearrange("p (f o) -> p f o", o=O)
        # PE warmup: dep-free matmuls over the bias tile keep the HAM
        # un-throttled through the input-load window.
        warm_ps = psum_pool.tile([128, nc_cols], F32, tag="ps", name="warm_ps")
        for _ in range(72):
            nc.tensor.matmul(
                warm_ps[0:8, 0:8],
                bias_sb[:, 0:8],
                bias_sb[:, 0:8],
                start=True,
                stop=True,
            )

        wt_sbs = [wt0_sb, wt1_sb, wt2_sb]

        def load_T(j):
            """x0t rows (40 f-slots, row 39 zero) broadcast to 128 partitions."""
            t_t = t_pool.tile([128, F_PAD, nc_cols], F8, tag="T", name=f"t_{j}")
            nc.sync.dma_start(t_t[:], xtc[j].partition_broadcast(128))
            return t_t

        def load_rhs0(j):
            """Host-packed folded-pair interaction products for layer 0."""
            r0 = rhs0_pool.tile([128, QG, nc_cols], F16, tag="r0", name=f"r0_{j}")
            nc.sync.dma_start(
                r0[:], rhs0c[j].rearrange("p (g i) -> p g i", i=nc_cols)
            )
            return r0

        def emit_mms(l, m, ps, rhs0, sl):
            if l == 0:
                for g in range(QG):
                    nc.tensor.matmul(
                        ps[:],
                        wt0_sb[:, g, m * 128 : (m + 1) * 128],
                        rhs0[:, g, :],
                        start=(g == 0),
                        stop=(g == QG - 1),
                    )
            else:
                wt_sb = wt_sbs[l]
                for s in range(NSL):
                    for t in range(PAIRS):
                        fp = s * SG + 2 * t
                        nc.tensor.matmul(
                            ps[:],
                            wt_sb[:, fp : fp + 2, m * 128 : (m + 1) * 128],
                            sl[s][:, 2 * t : 2 * t + 2, :],
                            start=(fp == 0),
                            stop=(fp == F_PAD - 2),
                            perf_mode=DR,
                        )

        def direct_out(j, l, ps, bias_col, osb):
            # one full-width relu+bias on ScalarE, one DVE segment-reduce over D
            dt = d_pool.tile([128, nc_cols], F16, tag="dt", name=f"dt_{j}_{bias_col}")
            nc.scalar.activation(
                dt[:],
                ps[:],
                AF.Relu,
                bias=bias_sb[:, bias_col : bias_col + 1],
                scale=(1.0 if l == 0 else 1.0 / WS),
            )
            nc.vector.tensor_reduce(
                osb[:, j * nb : (j + 1) * nb],
                dt[:].rearrange("p (b d) -> p b d", d=D),
                axis=mybir.AxisListType.X,
                op=mybir.AluOpType.add,
            )

        def tt_slices(j, l, newhid, t_t):
            new_slices = []
            for s in range(NSL):
                r_t = sl_pool.tile(
                    [128, SG, nc_cols], F8, tag="sl", name=f"sl_{j}_{l}_{s}"
                )
                in0b = newhid[:].unsqueeze(1).broadcast_to((128, SG, nc_cols))
                nc.vector.tensor_mul(r_t[:], in0b, t_t[:, s * SG : (s + 1) * SG, :])
                new_slices.append(r_t)
            return new_slices

        def l0_block(j, rhs0_t, t_t):
            """Emit L0(j) matmuls + hidden ACT + TT_L1(j) + direct epilogue."""
            ps1 = psum_pool.tile([128, nc_cols], F32, tag="ps", name=f"ps_{j}_0_1")
            emit_mms(0, 1, ps1, rhs0_t, None)
            h0 = hid_pool.tile([128, nc_cols], F16, tag="hid", name=f"hid_{j}_0")
            nc.scalar.activation(h0[:], ps1[:], AF.Relu, bias=bias_sb[:, 1:2])
            ps0 = psum_pool.tile([128, nc_cols], F32, tag="ps", name=f"ps_{j}_0_0")
            emit_mms(0, 0, ps0, rhs0_t, None)
            sl1 = tt_slices(j, 1, h0, t_t)
            direct_out(j, 0, ps0, 0, out_sb[0])
            return sl1

        # Rotated software pipeline. Steady-state PE stream per iteration k:
        #   L1m1(k) L1m0(k) | L0m1(k+1) L0m0(k+1) | L2m0(k) L2m1(k)
        # L2(k) sits between L0(k+1) and L1(k+1), so every ACT(hidden)+TT
        # chain has independent matmuls to hide behind.
        t_prev = load_T(0)
        rhs0_cur = load_rhs0(0)
        sl1_cur = l0_block(0, rhs0_cur, t_prev)

        for k in range(nchunk):
            if k == 0:
                for s in range(NSL):
                    nc.scalar.dma_start(
                        wt1_sb[:, s * SG : (s + 1) * SG, :],
                        wt1_r[:, s * SG : (s + 1) * SG, :],
                    )
            # prefetch chunk k+1 inputs
            if k + 1 < nchunk:
                t_cur = load_T(k + 1)
                rhs0_cur = load_rhs0(k + 1)

            # L1(k)
            ps1 = psum_pool.tile([128, nc_cols], F32, tag="ps", name=f"ps_{k}_1_1")
            emit_mms(1, 1, ps1, None, sl1_cur)
            h1 = hid_pool.tile([128, nc_cols], F16, tag="hid", name=f"hid_{k}_1")
            nc.scalar.activation(
                h1[:], ps1[:], AF.Relu, bias=bias_sb[:, 3:4], scale=1.0 / WS
            )
            ps0 = psum_pool.tile([128, nc_cols], F32, tag="ps", name=f"ps_{k}_1_0")
            emit_mms(1, 0, ps0, None, sl1_cur)
            sl2 = tt_slices(k, 2, h1, t_prev)
            direct_out(k, 1, ps0, 2, out_sb[1])

            if k == 0:
                for s in range(NSL):
                    nc.scalar.dma_start(
                        wt2_sb[:, s * SG : (s + 1) * SG, :],
                        wt2_r[:, s * SG : (s + 1) * SG, :],
                    )

            # L0(k+1) between L1(k) and L2(k)
            if k + 1 < nchunk:
                sl1_cur = l0_block(k + 1, rhs0_cur, t_cur)

            # L2(k)
            ps20 = psum_pool.tile([128, nc_cols], F32, tag="ps", name=f"ps_{k}_2_0")
            emit_mms(2, 0, ps20, None, sl2)
            ps21 = psum_pool.tile([128, nc_cols], F32, tag="ps", name=f"ps_{k}_2_1")
            emit_mms(2, 1, ps21, None, sl2)
            direct_out(k, 2, ps20, 4, out_sb[2])
            direct_out(k, 2, ps21, 5, out_sb[3])

            t_prev = t_cur if k + 1 < nchunk else None

        for i in range(4):
            nc.sync.dma_start(out[i], out_sb[i][:])

    nc.compile()
    return nc


def _to_e4(a):
    return np.clip(a, -240.0, 240.0).astype(E4NP)


def _pack_inputs(field_embeddings, w0, b0, w1, b1, w2, b2, b_loc=B_LOC, nc_cols=NC):
    """Host-side packing: shard x over cores, pre-transpose/convert weights."""
    x = np.asarray(field_embeddings, dtype=np.float32)
    w0 = np.asarray(w0, dtype=np.float32)
    w1 = np.asarray(w1, dtype=np.float32)
    w2 = np.asarray(w2, dtype=np.float32)
    ncores = x.shape[0] // b_loc
    n_loc = b_loc * D
    nchunk = n_loc // nc_cols

    # wt1/wt2 (fp8, x WS): [h, f*O + o] = w[o, h*39 + f] * WS, f-slot 39 zero
    def pack_w8(w):
        a = w.reshape(O, H, F).transpose(1, 2, 0)      # (h, f, o)
        ap = np.zeros((H, F_PAD, O), dtype=np.float32)
        ap[:, :F] = a * WS
        return _to_e4(ap.reshape(H, F_PAD * O))

    # wt0 (folded, fp16): pair q=(h<=f), row p, tile g with q = g*128+p;
    # Wf[o,q] = w0[o,h*39+f] + (h!=f)*w0[o,f*39+h]
    hq = np.array([h for f_ in range(F) for h in range(f_ + 1)])
    fq = np.array([f_ for f_ in range(F) for h in range(f_ + 1)])
    w0r = w0.reshape(O, F, F)
    wf = w0r[:, hq, fq] + np.where(hq == fq, 0.0, w0r[:, fq, hq])   # (O, NPAIR)
    wf_pad = np.zeros((O, Q), dtype=np.float32)
    wf_pad[:, :NPAIR] = wf
    wt0h = np.ascontiguousarray(
        wf_pad.reshape(O, QG, 128).transpose(2, 1, 0).reshape(128, QG * O)
    ).astype(np.float16)

    wt1h = pack_w8(w1)
    wt2h = pack_w8(w2)

    biash = np.zeros((128, 8), dtype=np.float32)
    for li, bvec in enumerate([b0, b1, b2]):
        bvec = np.asarray(bvec, dtype=np.float32)
        biash[:, 2 * li] = bvec[0:128]
        biash[:, 2 * li + 1] = bvec[128:256]

    in_maps = []
    for c in range(ncores):
        xc = x[c * b_loc : (c + 1) * b_loc]                  # (b_loc, F, D)
        x0t = xc.transpose(1, 0, 2).reshape(F, n_loc)        # (F, n_loc) fp32
        # T source: 40 f-slot rows in e4m3, row 39 zero
        xt_pad = np.zeros((F_PAD, n_loc), dtype=np.float32)
        xt_pad[:F] = x0t
        xtc_c = _to_e4(xt_pad).reshape(F_PAD, nchunk, nc_cols).transpose(1, 0, 2)
        # layer-0 rhs: folded products x_h*x_f rounded once to fp16,
        # device layout [j, p, g*nc+i] with pair row q = g*128+p
        prod = np.zeros((Q, n_loc), dtype=np.float16)
        prod[:NPAIR] = (x0t[hq] * x0t[fq]).astype(np.float16)
        r0 = prod.reshape(QG, 128, nchunk, nc_cols).transpose(2, 1, 0, 3)
        r0 = r0.reshape(nchunk, 128, QG * nc_cols)
        in_maps.append(
            {
                "xtc": np.ascontiguousarray(xtc_c),
                "rhs0c": np.ascontiguousarray(r0),
                "wt0": wt0h,
                "wt1": wt1h,
                "wt2": wt2h,
                "biases": biash,
            }
        )
    return in_maps


_MODULE = None


def kernel(field_embeddings, w0, b0, w1, b1, w2, b2):
    global _MODULE, _LAST_RESULTS
    if _MODULE is None:
        _MODULE = build_module()
    nc = _MODULE
    in_maps = _pack_inputs(field_embeddings, w0, b0, w1, b1, w2, b2)
    res = bass_utils.run_bass_kernel_spmd(
        nc, in_maps, core_ids=list(range(NCORES)), trace=TRACE
    )
    _LAST_RESULTS = res
    outs = []
    for c in range(NCORES):
        o = res.results[c]["out"]                  # (4, 128, B_LOC) fp32
        full = o.reshape(512, B_LOC)               # [L0;L1;L2a;L2b]
        outs.append(full.T)                        # (B_LOC, 512)
    return np.ascontiguousarray(np.concatenate(outs, axis=0), dtype=np.float32)


# revision 7
# speedup vs baseline: 1.5404x; 1.5404x over previous
"""CIN (Compressed Interaction Network) forward kernel for Trainium2.

Data-parallel over 8 NeuronCores: batch dim B=2048 is sharded 256/core,
conv weights are replicated. No cross-device communication.

Per-core layout: everything lives as (channels, n) where n = (b_local, d)
flattened to 8192 columns, processed in chunks of NC=512 columns.

Engine-balance design (measured rates):
  - DVE tensor_tensor: fp16 2x_1p mode = 0.53 ns/elem; any fp8 operand
    drops to 1x = 1.05 ns/elem (TT has no uops above 2x_1p, which
    requires 16-bit dtypes).
  - PE: fp16 matmul 1 row/cycle; fp8 e4m3 DoubleRow = 2 K-rows/cycle.
  So per f-slot, fp16 costs 0.53/elem DVE + 0.86/elem PE while fp8
  costs 1.05/elem DVE + 0.43/elem PE. Splitting each layer's 39 f-slots
  into N16=22 fp16 slots + N8=18 fp8 slots (9 DoubleRow pairs, one zero
  pad) balances DVE and PE at ~31 us/chunk.
  - Layer 0 stays fp16 for accuracy (its hidden feeds layers 1/2); its
    folded x (x) x interaction products are precomputed on host and
    DMA'd directly (no DVE work).
All conv weights are host-scaled by 64 (exact in fp16, lands fp8 e4m3
in its normal range); ScalarE activations apply 1/64. ReLU + bias are
fused into ScalarE; sum-over-D runs as DVE segment-reduce.
"""

import sys

if "/opt/trn_rl_repo" not in sys.path:
    sys.path.insert(0, "/opt/trn_rl_repo")

from contextlib import ExitStack

import ml_dtypes
import numpy as np

import concourse.bacc as bacc
import concourse.bass as bass
import concourse.mybir as mybir
import concourse.tile as tile
from concourse import bass_utils

# Problem shapes (hardcoded per contest rules)
B, F, D = 2048, 39, 32
O = 256          # conv output channels per layer
H = 128          # hidden channels fed to layers 1,2
NCORES = 8
B_LOC = B // NCORES          # 256 batches per core
N_LOC = B_LOC * D            # 8192 columns per core

NC = 512                     # columns per chunk
NB = NC // D                 # batches per chunk (16)
N16 = 22                     # fp16 f-slots per layer (f = 0..21)
N8 = 18                      # fp8 f-slots (f = 22..38 + zero pad), 9 DR pairs
PAIRS = N8 // 2              # 9
S16 = N16 // 2               # fp16 build-slice size (11)
S8A, S8B = 10, 8             # fp8 build-slice sizes (pairs never straddle)
# layer-0 symmetry folding: x0 (x) x0 is symmetric, keep pairs h <= f only
NPAIR = F * (F + 1) // 2     # 780
QG = (NPAIR + 127) // 128    # 7 K-tiles
Q = QG * 128                 # 896 padded rows
WS = 64.0                    # weight pre-scale for layers 1/2 (undone in act)

F8 = mybir.dt.float8e4
F16 = mybir.dt.float16
F32 = mybir.dt.float32
AF = mybir.ActivationFunctionType
DR = mybir.MatmulPerfMode.DoubleRow
E4NP = ml_dtypes.float8_e4m3

TRACE = False                # set True from test harness to profile
_LAST_RESULTS = None         # BassKernelResults of last run (for test.py)


def build_module(b_loc=B_LOC, nc_cols=NC):
    """Build the Bass/Tile module for one core (shapes are per-core)."""
    n_loc = b_loc * D
    nchunk = n_loc // nc_cols
    nb = nc_cols // D
    assert n_loc % nc_cols == 0 and nc_cols % D == 0

    nc = bacc.Bacc("TRN2", target_bir_lowering=False, debug=False)

    xt16 = nc.dram_tensor("xt16", (nchunk, N16, nc_cols), F16, kind="ExternalInput").ap()
    xt8 = nc.dram_tensor("xt8", (nchunk, N8, nc_cols), F8, kind="ExternalInput").ap()
    rhs0c = nc.dram_tensor(
        "rhs0c", (nchunk, 128, QG * nc_cols), F16, kind="ExternalInput"
    ).ap()
    wt0 = nc.dram_tensor("wt0", (128, QG * O), F16, kind="ExternalInput").ap()
    wa1 = nc.dram_tensor("wa1", (128, N16 * O), F16, kind="ExternalInput").ap()
    wb1 = nc.dram_tensor("wb1", (128, N8 * O), F8, kind="ExternalInput").ap()
    wa2 = nc.dram_tensor("wa2", (128, N16 * O), F16, kind="ExternalInput").ap()
    wb2 = nc.dram_tensor("wb2", (128, N8 * O), F8, kind="ExternalInput").ap()
    biases = nc.dram_tensor("biases", (128, 8), F32, kind="ExternalInput").ap()
    out = nc.dram_tensor("out", (4, 128, b_loc), F32, kind="ExternalOutput").ap()

    with tile.TileContext(nc) as tc, ExitStack() as ctx:
        const = ctx.enter_context(tc.tile_pool(name="const", bufs=1))
        t_pool = ctx.enter_context(tc.tile_pool(name="tpool", bufs=2))
        rhs0_pool = ctx.enter_context(tc.tile_pool(name="r0pool", bufs=2))
        sl_pool = ctx.enter_context(tc.tile_pool(name="slpool", bufs=4))
        hid_pool = ctx.enter_context(tc.tile_pool(name="hidpool", bufs=4))
        d_pool = ctx.enter_context(tc.tile_pool(name="dpool", bufs=3))
        psum_pool = ctx.enter_context(tc.tile_pool(name="psum", bufs=8, space="PSUM"))

        # --- resident tensors ---
        wt0_sb = const.tile([128, QG, O], F16)
        wa_sb = [const.tile([128, N16, O], F16, name=f"wa{l}") for l in (1, 2)]
        wb_sb = [const.tile([128, N8, O], F8, name=f"wb{l}") for l in (1, 2)]
        bias_sb = const.tile([128, 8], F32)
        out_sb = [const.tile([128, b_loc], F32, name=f"osb{i}") for i in range(4)]

        # Preamble DMAs: only what chunk 0's layer 0 needs, in consumption
        # order on the SP ring. Layer-1/2 weights are emitted lazily (on the
        # ACT HWDGE ring) right before their first consumers.
        nc.sync.dma_start(bias_sb[:], biases)
        nc.sync.dma_start(wt0_sb[:], wt0.rearrange("p (g o) -> p g o", o=O))
        wa_r = [w.rearrange("p (f o) -> p f o", o=O) for w in (wa1, wa2)]
        wb_r = [w.rearrange("p (f o) -> p f o", o=O) for w in (wb1, wb2)]
        # PE warmup: dep-free matmuls over the bias tile keep the HAM
        # un-throttled through the input-load window.
        warm_ps = psum_pool.tile([128, nc_cols], F32, tag="ps", name="warm_ps")
        for _ in range(72):
            nc.tensor.matmul(
                warm_ps[0:8, 0:8],
                bias_sb[:, 0:8],
                bias_sb[:, 0:8],
                start=True,
                stop=True,
            )

        def load_T(j):
            """x0t rows broadcast to 128 partitions: fp16 + fp8 slot groups."""
            t16 = t_pool.tile([128, N16, nc_cols], F16, tag="T16", name=f"t16_{j}")
            nc.sync.dma_start(t16[:], xt16[j].partition_broadcast(128))
            t8 = t_pool.tile([128, N8, nc_cols], F8, tag="T8", name=f"t8_{j}")
            nc.sync.dma_start(t8[:], xt8[j].partition_broadcast(128))
            return t16, t8

        def load_rhs0(j):
            """Host-packed folded-pair interaction products for layer 0."""
            r0 = rhs0_pool.tile([128, QG, nc_cols], F16, tag="r0", name=f"r0_{j}")
            nc.sync.dma_start(
                r0[:], rhs0c[j].rearrange("p (g i) -> p g i", i=nc_cols)
            )
            return r0

        def emit_mms(l, m, ps, rhs0, sl):
            if l == 0:
                for g in range(QG):
                    nc.tensor.matmul(
                        ps[:],
                        wt0_sb[:, g, m * 128 : (m + 1) * 128],
                        rhs0[:, g, :],
                        start=(g == 0),
                        stop=(g == QG - 1),
                    )
                return
            wa, wb = wa_sb[l - 1], wb_sb[l - 1]
            sl16a, sl16b, sl8a, sl8b = sl
            for f in range(N16):
                st = sl16a if f < S16 else sl16b
                nc.tensor.matmul(
                    ps[:],
                    wa[:, f, m * 128 : (m + 1) * 128],
                    st[:, f % S16, :],
                    start=(f == 0),
                    stop=False,
                )
            for t in range(PAIRS):
                fp = 2 * t
                st = sl8a if fp < S8A else sl8b
                o = fp if fp < S8A else fp - S8A
                nc.tensor.matmul(
                    ps[:],
                    wb[:, fp : fp + 2, m * 128 : (m + 1) * 128],
                    st[:, o : o + 2, :],
                    start=False,
                    stop=(t == PAIRS - 1),
                    perf_mode=DR,
                )

        def direct_out(j, l, ps, bias_col, osb):
            # one full-width relu+bias on ScalarE, one DVE segment-reduce over D
            dt = d_pool.tile([128, nc_cols], F16, tag="dt", name=f"dt_{j}_{bias_col}")
            nc.scalar.activation(
                dt[:],
                ps[:],
                AF.Relu,
                bias=bias_sb[:, bias_col : bias_col + 1],
                scale=(1.0 if l == 0 else 1.0 / WS),
            )
            nc.vector.tensor_reduce(
                osb[:, j * nb : (j + 1) * nb],
                dt[:].rearrange("p (b d) -> p b d", d=D),
                axis=mybir.AxisListType.X,
                op=mybir.AluOpType.add,
            )

        def tt_slices(j, l, newhid, t16, t8):
            """rhs products: 2 fp16 slices then 2 fp8 slices (PE consumes in
            the same order, so the first matmul only waits for slice one)."""
            outs = []
            for s, (lo, width, dtype, t_t, tag) in enumerate(
                [
                    (0, S16, F16, t16, "s16"),
                    (S16, S16, F16, t16, "s16"),
                    (0, S8A, F8, t8, "s8a"),
                    (S8A, S8B, F8, t8, "s8b"),
                ]
            ):
                r = sl_pool.tile(
                    [128, width, nc_cols], dtype, tag=tag, name=f"sl_{j}_{l}_{s}"
                )
                in0b = newhid[:].unsqueeze(1).broadcast_to((128, width, nc_cols))
                nc.vector.tensor_mul(r[:], in0b, t_t[:, lo : lo + width, :])
                outs.append(r)
            return outs

        def l0_block(j, rhs0_t, t16, t8):
            """Emit L0(j) matmuls + hidden ACT + TT_L1(j) + direct epilogue."""
            ps1 = psum_pool.tile([128, nc_cols], F32, tag="ps", name=f"ps_{j}_0_1")
            emit_mms(0, 1, ps1, rhs0_t, None)
            h0 = hid_pool.tile([128, nc_cols], F16, tag="hid", name=f"hid_{j}_0")
            nc.scalar.activation(h0[:], ps1[:], AF.Relu, bias=bias_sb[:, 1:2])
            ps0 = psum_pool.tile([128, nc_cols], F32, tag="ps", name=f"ps_{j}_0_0")
            emit_mms(0, 0, ps0, rhs0_t, None)
            sl1 = tt_slices(j, 1, h0, t16, t8)
            direct_out(j, 0, ps0, 0, out_sb[0])
            return sl1

        # Rotated software pipeline. Steady-state PE stream per iteration k:
        #   L1m1(k) L1m0(k) | L0m1(k+1) L0m0(k+1) | L2m0(k) L2m1(k)
        # L2(k) sits between L0(k+1) and L1(k+1), so every ACT(hidden)+TT
        # chain has independent matmuls to hide behind.
        t_prev = load_T(0)
        rhs0_cur = load_rhs0(0)
        sl1_cur = l0_block(0, rhs0_cur, *t_prev)

        for k in range(nchunk):
            if k == 0:
                nc.scalar.dma_start(wa_sb[0][:], wa_r[0])
                nc.scalar.dma_start(wb_sb[0][:], wb_r[0])
            # prefetch chunk k+1 inputs
            if k + 1 < nchunk:
                t_cur = load_T(k + 1)
                rhs0_cur = load_rhs0(k + 1)

            # L1(k)
            ps1 = psum_pool.tile([128, nc_cols], F32, tag="ps", name=f"ps_{k}_1_1")
            emit_mms(1, 1, ps1, None, sl1_cur)
            h1 = hid_pool.tile([128, nc_cols], F16, tag="hid", name=f"hid_{k}_1")
            nc.scalar.activation(
                h1[:], ps1[:], AF.Relu, bias=bias_sb[:, 3:4], scale=1.0 / WS
            )
            ps0 = psum_pool.tile([128, nc_cols], F32, tag="ps", name=f"ps_{k}_1_0")
            emit_mms(1, 0, ps0, None, sl1_cur)
            sl2 = tt_slices(k, 2, h1, *t_prev)
            direct_out(k, 1, ps0, 2, out_sb[1])

            if k == 0:
                nc.scalar.dma_start(wa_sb[1][:], wa_r[1])
                nc.scalar.dma_start(wb_sb[1][:], wb_r[1])

            # L0(k+1) between L1(k) and L2(k)
            if k + 1 < nchunk:
                sl1_cur = l0_block(k + 1, rhs0_cur, *t_cur)

            # L2(k)
            ps20 = psum_pool.tile([128, nc_cols], F32, tag="ps", name=f"ps_{k}_2_0")
            emit_mms(2, 0, ps20, None, sl2)
            ps21 = psum_pool.tile([128, nc_cols], F32, tag="ps", name=f"ps_{k}_2_1")
            emit_mms(2, 1, ps21, None, sl2)
            direct_out(k, 2, ps20, 4, out_sb[2])
            direct_out(k, 2, ps21, 5, out_sb[3])

            t_prev = t_cur if k + 1 < nchunk else None

        for i in range(4):
            nc.sync.dma_start(out[i], out_sb[i][:])

    nc.compile()
    return nc


def _to_e4(a):
    return np.clip(a, -240.0, 240.0).astype(E4NP)


def _pack_inputs(field_embeddings, w0, b0, w1, b1, w2, b2, b_loc=B_LOC, nc_cols=NC):
    """Host-side packing: shard x over cores, pre-transpose/convert weights."""
    x = np.asarray(field_embeddings, dtype=np.float32)
    w0 = np.asarray(w0, dtype=np.float32)
    w1 = np.asarray(w1, dtype=np.float32)
    w2 = np.asarray(w2, dtype=np.float32)
    ncores = x.shape[0] // b_loc
    n_loc = b_loc * D
    nchunk = n_loc // nc_cols

    # layers 1/2: [h, f, o] = w[o, h*39 + f] * WS; fp16 group f<N16,
    # fp8 group f in [N16, 39) plus one zero pad slot
    def pack_w12(w):
        a = w.reshape(O, H, F).transpose(1, 2, 0) * WS   # (h, f, o)
        wa = np.ascontiguousarray(a[:, :N16]).reshape(H, N16 * O).astype(np.float16)
        bp = np.zeros((H, N8, O), dtype=np.float32)
        bp[:, : F - N16] = a[:, N16:]
        return wa, _to_e4(bp.reshape(H, N8 * O))

    # wt0 (folded, fp16): pair q=(h<=f), row p, tile g with q = g*128+p;
    # Wf[o,q] = w0[o,h*39+f] + (h!=f)*w0[o,f*39+h]
    hq = np.array([h for f_ in range(F) for h in range(f_ + 1)])
    fq = np.array([f_ for f_ in range(F) for h in range(f_ + 1)])
    w0r = w0.reshape(O, F, F)
    wf = w0r[:, hq, fq] + np.where(hq == fq, 0.0, w0r[:, fq, hq])   # (O, NPAIR)
    wf_pad = np.zeros((O, Q), dtype=np.float32)
    wf_pad[:, :NPAIR] = wf
    wt0h = np.ascontiguousarray(
        wf_pad.reshape(O, QG, 128).transpose(2, 1, 0).reshape(128, QG * O)
    ).astype(np.float16)

    wa1h, wb1h = pack_w12(w1)
    wa2h, wb2h = pack_w12(w2)

    biash = np.zeros((128, 8), dtype=np.float32)
    for li, bvec in enumerate([b0, b1, b2]):
        bvec = np.asarray(bvec, dtype=np.float32)
        biash[:, 2 * li] = bvec[0:128]
        biash[:, 2 * li + 1] = bvec[128:256]

    in_maps = []
    for c in range(ncores):
        xc = x[c * b_loc : (c + 1) * b_loc]                  # (b_loc, F, D)
        x0t = xc.transpose(1, 0, 2).reshape(F, n_loc)        # (F, n_loc) fp32
        xt16c = (
            x0t[:N16].astype(np.float16).reshape(N16, nchunk, nc_cols)
            .transpose(1, 0, 2)
        )
        x8_pad = np.zeros((N8, n_loc), dtype=np.float32)
        x8_pad[: F - N16] = x0t[N16:]
        xt8c = _to_e4(x8_pad).reshape(N8, nchunk, nc_cols).transpose(1, 0, 2)
        # layer-0 rhs: folded products x_h*x_f rounded once to fp16,
        # device layout [j, p, g*nc+i] with pair row q = g*128+p
        prod = np.zeros((Q, n_loc), dtype=np.float16)
        prod[:NPAIR] = (x0t[hq] * x0t[fq]).astype(np.float16)
        r0 = prod.reshape(QG, 128, nchunk, nc_cols).transpose(2, 1, 0, 3)
        r0 = r0.reshape(nchunk, 128, QG * nc_cols)
        in_maps.append(
            {
                "xt16": np.ascontiguousarray(xt16c),
                "xt8": np.ascontiguousarray(xt8c),
                "rhs0c": np.ascontiguousarray(r0),
                "wt0": wt0h,
                "wa1": wa1h,
                "wb1": wb1h,
                "wa2": wa2h,
                "wb2": wb2h,
                "biases": biash,
            }
        )
    return in_maps


_MODULE = None


def kernel(field_embeddings, w0, b0, w1, b1, w2, b2):
    global _MODULE, _LAST_RESULTS
    if _MODULE is None:
        _MODULE = build_module()
    nc = _MODULE
    in_maps = _pack_inputs(field_embeddings, w0, b0, w1, b1, w2, b2)
    res = bass_utils.run_bass_kernel_spmd(
        nc, in_maps, core_ids=list(range(NCORES)), trace=TRACE
    )
    _LAST_RESULTS = res
    outs = []
    for c in range(NCORES):
        o = res.results[c]["out"]                  # (4, 128, B_LOC) fp32
        full = o.reshape(512, B_LOC)               # [L0;L1;L2a;L2b]
        outs.append(full.T)                        # (B_LOC, 512)
    return np.ascontiguousarray(np.concatenate(outs, axis=0), dtype=np.float32)


# revision 8
# speedup vs baseline: 1.6106x; 1.0456x over previous
"""CIN (Compressed Interaction Network) forward kernel for Trainium2.

Data-parallel over 8 NeuronCores: batch dim B=2048 is sharded 256/core,
conv weights are replicated. No cross-device communication.

Per-core layout: everything lives as (channels, n) where n = (b_local, d)
flattened to 8192 columns, processed in chunks of NC=512 columns.

Engine-balance design (measured rates):
  - DVE tensor_tensor: fp16 2x_1p mode = 0.53 ns/elem; any fp8 operand
    drops to 1x = 1.05 ns/elem (TT has no uops above 2x_1p, which
    requires 16-bit dtypes).
  - PE: fp16 matmul 1 row/cycle; fp8 e4m3 DoubleRow = 2 K-rows/cycle.
  So per f-slot, fp16 costs 0.53/elem DVE + 0.86/elem PE while fp8
  costs 1.05/elem DVE + 0.43/elem PE. Splitting each layer's 39 f-slots
  into N16=22 fp16 slots + N8=18 fp8 slots (9 DoubleRow pairs, one zero
  pad) balances DVE and PE at ~31 us/chunk.
  - Layer 0 stays fp16 for accuracy (its hidden feeds layers 1/2); its
    folded x (x) x interaction products are precomputed on host and
    DMA'd directly (no DVE work).
All conv weights are host-scaled by 64 (exact in fp16, lands fp8 e4m3
in its normal range); ScalarE activations apply 1/64. ReLU + bias are
fused into ScalarE; sum-over-D runs as DVE segment-reduce.
"""

import sys

if "/opt/trn_rl_repo" not in sys.path:
    sys.path.insert(0, "/opt/trn_rl_repo")

from contextlib import ExitStack

import ml_dtypes
import numpy as np

import concourse.bacc as bacc
import concourse.bass as bass
import concourse.mybir as mybir
import concourse.tile as tile
from concourse import bass_utils

# Problem shapes (hardcoded per contest rules)
B, F, D = 2048, 39, 32
O = 256          # conv output channels per layer
H = 128          # hidden channels fed to layers 1,2
NCORES = 8
B_LOC = B // NCORES          # 256 batches per core
N_LOC = B_LOC * D            # 8192 columns per core

NC = 512                     # columns per chunk
NB = NC // D                 # batches per chunk (16)
N16 = 23                     # fp16 f-slots per layer (f = 0..22)
N8 = 16                      # fp8 f-slots (f = 23..38), 8 DR pairs, no pad
PAIRS = N8 // 2              # 8
S16A, S16B = 12, 11          # fp16 build-slice sizes
S8A, S8B = 8, 8              # fp8 build-slice sizes (pairs never straddle)
# layer-0 symmetry folding: x0 (x) x0 is symmetric, keep pairs h <= f only
NPAIR = F * (F + 1) // 2     # 780
QG = (NPAIR + 127) // 128    # 7 K-tiles
Q = QG * 128                 # 896 padded rows
WS = 64.0                    # weight pre-scale for layers 1/2 (undone in act)

F8 = mybir.dt.float8e4
F16 = mybir.dt.float16
F32 = mybir.dt.float32
AF = mybir.ActivationFunctionType
DR = mybir.MatmulPerfMode.DoubleRow
E4NP = ml_dtypes.float8_e4m3

TRACE = False                # set True from test harness to profile
_LAST_RESULTS = None         # BassKernelResults of last run (for test.py)


def build_module(b_loc=B_LOC, nc_cols=NC):
    """Build the Bass/Tile module for one core (shapes are per-core)."""
    n_loc = b_loc * D
    nchunk = n_loc // nc_cols
    nb = nc_cols // D
    assert n_loc % nc_cols == 0 and nc_cols % D == 0

    nc = bacc.Bacc("TRN2", target_bir_lowering=False, debug=False)

    xt16 = nc.dram_tensor("xt16", (nchunk, N16, nc_cols), F16, kind="ExternalInput").ap()
    xt8 = nc.dram_tensor("xt8", (nchunk, N8, nc_cols), F8, kind="ExternalInput").ap()
    rhs0c = nc.dram_tensor(
        "rhs0c", (nchunk, 128, QG * nc_cols), F16, kind="ExternalInput"
    ).ap()
    wt0 = nc.dram_tensor("wt0", (128, QG * O), F16, kind="ExternalInput").ap()
    wa1 = nc.dram_tensor("wa1", (128, N16 * O), F16, kind="ExternalInput").ap()
    wb1 = nc.dram_tensor("wb1", (128, N8 * O), F8, kind="ExternalInput").ap()
    wa2 = nc.dram_tensor("wa2", (128, N16 * O), F16, kind="ExternalInput").ap()
    wb2 = nc.dram_tensor("wb2", (128, N8 * O), F8, kind="ExternalInput").ap()
    biases = nc.dram_tensor("biases", (128, 8), F32, kind="ExternalInput").ap()
    out = nc.dram_tensor("out", (4, 128, b_loc), F32, kind="ExternalOutput").ap()

    with tile.TileContext(nc) as tc, ExitStack() as ctx:
        const = ctx.enter_context(tc.tile_pool(name="const", bufs=1))
        t_pool = ctx.enter_context(tc.tile_pool(name="tpool", bufs=2))
        rhs0_pool = ctx.enter_context(tc.tile_pool(name="r0pool", bufs=2))
        sl_pool = ctx.enter_context(tc.tile_pool(name="slpool", bufs=4))
        hid_pool = ctx.enter_context(tc.tile_pool(name="hidpool", bufs=4))
        d_pool = ctx.enter_context(tc.tile_pool(name="dpool", bufs=3))
        psum_pool = ctx.enter_context(tc.tile_pool(name="psum", bufs=8, space="PSUM"))

        # --- resident tensors ---
        wt0_sb = const.tile([128, QG, O], F16)
        wa_sb = [const.tile([128, N16, O], F16, name=f"wa{l}") for l in (1, 2)]
        wb_sb = [const.tile([128, N8, O], F8, name=f"wb{l}") for l in (1, 2)]
        bias_sb = const.tile([128, 8], F32)
        out_sb = [const.tile([128, b_loc], F32, name=f"osb{i}") for i in range(4)]

        # Preamble DMAs: only what chunk 0's layer 0 needs, in consumption
        # order on the SP ring. Layer-1/2 weights are emitted lazily (on the
        # ACT HWDGE ring) right before their first consumers.
        nc.sync.dma_start(bias_sb[:], biases)
        nc.sync.dma_start(wt0_sb[:], wt0.rearrange("p (g o) -> p g o", o=O))
        wa_r = [w.rearrange("p (f o) -> p f o", o=O) for w in (wa1, wa2)]
        wb_r = [w.rearrange("p (f o) -> p f o", o=O) for w in (wb1, wb2)]
        # PE warmup: dep-free matmuls over the bias tile keep the HAM
        # un-throttled through the input-load window.
        warm_ps = psum_pool.tile([128, nc_cols], F32, tag="ps", name="warm_ps")
        for _ in range(72):
            nc.tensor.matmul(
                warm_ps[0:8, 0:8],
                bias_sb[:, 0:8],
                bias_sb[:, 0:8],
                start=True,
                stop=True,
            )

        def load_T(j):
            """x0t rows broadcast to 128 partitions: fp16 + fp8 slot groups."""
            t16 = t_pool.tile([128, N16, nc_cols], F16, tag="T16", name=f"t16_{j}")
            nc.sync.dma_start(t16[:], xt16[j].partition_broadcast(128))
            t8 = t_pool.tile([128, N8, nc_cols], F8, tag="T8", name=f"t8_{j}")
            nc.sync.dma_start(t8[:], xt8[j].partition_broadcast(128))
            return t16, t8

        def load_rhs0(j):
            """Host-packed folded-pair interaction products for layer 0."""
            r0 = rhs0_pool.tile([128, QG, nc_cols], F16, tag="r0", name=f"r0_{j}")
            nc.sync.dma_start(
                r0[:], rhs0c[j].rearrange("p (g i) -> p g i", i=nc_cols)
            )
            return r0

        def emit_mms(l, m, ps, rhs0, sl):
            if l == 0:
                for g in range(QG):
                    nc.tensor.matmul(
                        ps[:],
                        wt0_sb[:, g, m * 128 : (m + 1) * 128],
                        rhs0[:, g, :],
                        start=(g == 0),
                        stop=(g == QG - 1),
                    )
                return
            wa, wb = wa_sb[l - 1], wb_sb[l - 1]
            sl16a, sl16b, sl8a, sl8b = sl
            for f in range(N16):
                st = sl16a if f < S16A else sl16b
                nc.tensor.matmul(
                    ps[:],
                    wa[:, f, m * 128 : (m + 1) * 128],
                    st[:, f if f < S16A else f - S16A, :],
                    start=(f == 0),
                    stop=False,
                )
            for t in range(PAIRS):
                fp = 2 * t
                st = sl8a if fp < S8A else sl8b
                o = fp if fp < S8A else fp - S8A
                nc.tensor.matmul(
                    ps[:],
                    wb[:, fp : fp + 2, m * 128 : (m + 1) * 128],
                    st[:, o : o + 2, :],
                    start=False,
                    stop=(t == PAIRS - 1),
                    perf_mode=DR,
                )

        def direct_out(j, l, ps, bias_col, osb):
            # relu+bias+sum-over-D fused on ScalarE (accum_out), one ACT per
            # batch column; keeps the D-sums off the DVE critical path
            scale = 1.0 if l == 0 else 1.0 / WS
            dt = d_pool.tile([128, D], F16, tag="dt", name=f"dt_{j}_{bias_col}")
            for i in range(nb):
                c = j * nb + i
                nc.scalar.activation(
                    dt[:],
                    ps[:, i * D : (i + 1) * D],
                    AF.Relu,
                    bias=bias_sb[:, bias_col : bias_col + 1],
                    scale=scale,
                    accum_out=osb[:, c : c + 1],
                )

        def tt_slices(j, l, newhid, t16, t8):
            """rhs products: 2 fp16 slices then 2 fp8 slices (PE consumes in
            the same order, so the first matmul only waits for slice one)."""
            outs = []
            for s, (lo, width, dtype, t_t, tag) in enumerate(
                [
                    (0, S16A, F16, t16, "s16"),
                    (S16A, S16B, F16, t16, "s16"),
                    (0, S8A, F8, t8, "s8a"),
                    (S8A, S8B, F8, t8, "s8b"),
                ]
            ):
                r = sl_pool.tile(
                    [128, width, nc_cols], dtype, tag=tag, name=f"sl_{j}_{l}_{s}"
                )
                in0b = newhid[:].unsqueeze(1).broadcast_to((128, width, nc_cols))
                nc.vector.tensor_mul(r[:], in0b, t_t[:, lo : lo + width, :])
                outs.append(r)
            return outs

        def l0_block(j, rhs0_t, t16, t8):
            """Emit L0(j) matmuls + hidden ACT + TT_L1(j) + direct epilogue."""
            ps1 = psum_pool.tile([128, nc_cols], F32, tag="ps", name=f"ps_{j}_0_1")
            emit_mms(0, 1, ps1, rhs0_t, None)
            h0 = hid_pool.tile([128, nc_cols], F16, tag="hid", name=f"hid_{j}_0")
            nc.scalar.activation(h0[:], ps1[:], AF.Relu, bias=bias_sb[:, 1:2])
            ps0 = psum_pool.tile([128, nc_cols], F32, tag="ps", name=f"ps_{j}_0_0")
            emit_mms(0, 0, ps0, rhs0_t, None)
            sl1 = tt_slices(j, 1, h0, t16, t8)
            direct_out(j, 0, ps0, 0, out_sb[0])
            return sl1

        # Rotated software pipeline. Steady-state PE stream per iteration k:
        #   L1m1(k) L1m0(k) | L0m1(k+1) L0m0(k+1) | L2m0(k) L2m1(k)
        # L2(k) sits between L0(k+1) and L1(k+1), so every ACT(hidden)+TT
        # chain has independent matmuls to hide behind.
        t_prev = load_T(0)
        rhs0_cur = load_rhs0(0)
        sl1_cur = l0_block(0, rhs0_cur, *t_prev)

        for k in range(nchunk):
            if k == 0:
                nc.scalar.dma_start(wa_sb[0][:], wa_r[0])
                nc.scalar.dma_start(wb_sb[0][:], wb_r[0])
            # prefetch chunk k+1 inputs
            if k + 1 < nchunk:
                t_cur = load_T(k + 1)
                rhs0_cur = load_rhs0(k + 1)

            # L1(k)
            ps1 = psum_pool.tile([128, nc_cols], F32, tag="ps", name=f"ps_{k}_1_1")
            emit_mms(1, 1, ps1, None, sl1_cur)
            h1 = hid_pool.tile([128, nc_cols], F16, tag="hid", name=f"hid_{k}_1")
            nc.scalar.activation(
                h1[:], ps1[:], AF.Relu, bias=bias_sb[:, 3:4], scale=1.0 / WS
            )
            ps0 = psum_pool.tile([128, nc_cols], F32, tag="ps", name=f"ps_{k}_1_0")
            emit_mms(1, 0, ps0, None, sl1_cur)
            sl2 = tt_slices(k, 2, h1, *t_prev)
            direct_out(k, 1, ps0, 2, out_sb[1])

            if k == 0:
                nc.scalar.dma_start(wa_sb[1][:], wa_r[1])
                nc.scalar.dma_start(wb_sb[1][:], wb_r[1])

            # L0(k+1) between L1(k) and L2(k)
            if k + 1 < nchunk:
                sl1_cur = l0_block(k + 1, rhs0_cur, *t_cur)

            # L2(k)
            ps20 = psum_pool.tile([128, nc_cols], F32, tag="ps", name=f"ps_{k}_2_0")
            emit_mms(2, 0, ps20, None, sl2)
            ps21 = psum_pool.tile([128, nc_cols], F32, tag="ps", name=f"ps_{k}_2_1")
            emit_mms(2, 1, ps21, None, sl2)
            direct_out(k, 2, ps20, 4, out_sb[2])
            direct_out(k, 2, ps21, 5, out_sb[3])

            t_prev = t_cur if k + 1 < nchunk else None

        for i in range(4):
            nc.sync.dma_start(out[i], out_sb[i][:])

    nc.compile()
    return nc


def _to_e4(a):
    return np.clip(a, -240.0, 240.0).astype(E4NP)


def _pack_inputs(field_embeddings, w0, b0, w1, b1, w2, b2, b_loc=B_LOC, nc_cols=NC):
    """Host-side packing: shard x over cores, pre-transpose/convert weights."""
    x = np.asarray(field_embeddings, dtype=np.float32)
    w0 = np.asarray(w0, dtype=np.float32)
    w1 = np.asarray(w1, dtype=np.float32)
    w2 = np.asarray(w2, dtype=np.float32)
    ncores = x.shape[0] // b_loc
    n_loc = b_loc * D
    nchunk = n_loc // nc_cols

    # layers 1/2: [h, f, o] = w[o, h*39 + f] * WS; fp16 group f<N16,
    # fp8 group f in [N16, 39) plus one zero pad slot
    def pack_w12(w):
        a = w.reshape(O, H, F).transpose(1, 2, 0) * WS   # (h, f, o)
        wa = np.ascontiguousarray(a[:, :N16]).reshape(H, N16 * O).astype(np.float16)
        bp = np.ascontiguousarray(a[:, N16:]).reshape(H, N8 * O)
        return wa, _to_e4(bp)

    # wt0 (folded, fp16): pair q=(h<=f), row p, tile g with q = g*128+p;
    # Wf[o,q] = w0[o,h*39+f] + (h!=f)*w0[o,f*39+h]
    hq = np.array([h for f_ in range(F) for h in range(f_ + 1)])
    fq = np.array([f_ for f_ in range(F) for h in range(f_ + 1)])
    w0r = w0.reshape(O, F, F)
    wf = w0r[:, hq, fq] + np.where(hq == fq, 0.0, w0r[:, fq, hq])   # (O, NPAIR)
    wf_pad = np.zeros((O, Q), dtype=np.float32)
    wf_pad[:, :NPAIR] = wf
    wt0h = np.ascontiguousarray(
        wf_pad.reshape(O, QG, 128).transpose(2, 1, 0).reshape(128, QG * O)
    ).astype(np.float16)

    wa1h, wb1h = pack_w12(w1)
    wa2h, wb2h = pack_w12(w2)

    biash = np.zeros((128, 8), dtype=np.float32)
    for li, bvec in enumerate([b0, b1, b2]):
        bvec = np.asarray(bvec, dtype=np.float32)
        biash[:, 2 * li] = bvec[0:128]
        biash[:, 2 * li + 1] = bvec[128:256]

    in_maps = []
    for c in range(ncores):
        xc = x[c * b_loc : (c + 1) * b_loc]                  # (b_loc, F, D)
        x0t = xc.transpose(1, 0, 2).reshape(F, n_loc)        # (F, n_loc) fp32
        xt16c = (
            x0t[:N16].astype(np.float16).reshape(N16, nchunk, nc_cols)
            .transpose(1, 0, 2)
        )
        xt8c = _to_e4(x0t[N16:]).reshape(N8, nchunk, nc_cols).transpose(1, 0, 2)
        # layer-0 rhs: folded products x_h*x_f rounded once to fp16,
        # device layout [j, p, g*nc+i] with pair row q = g*128+p
        prod = np.zeros((Q, n_loc), dtype=np.float16)
        prod[:NPAIR] = (x0t[hq] * x0t[fq]).astype(np.float16)
        r0 = prod.reshape(QG, 128, nchunk, nc_cols).transpose(2, 1, 0, 3)
        r0 = r0.reshape(nchunk, 128, QG * nc_cols)
        in_maps.append(
            {
                "xt16": np.ascontiguousarray(xt16c),
                "xt8": np.ascontiguousarray(xt8c),
                "rhs0c": np.ascontiguousarray(r0),
                "wt0": wt0h,
                "wa1": wa1h,
                "wb1": wb1h,
                "wa2": wa2h,
                "wb2": wb2h,
                "biases": biash,
            }
        )
    return in_maps


_MODULE = None


def kernel(field_embeddings, w0, b0, w1, b1, w2, b2):
    global _MODULE, _LAST_RESULTS
    if _MODULE is None:
        _MODULE = build_module()
    nc = _MODULE
    in_maps = _pack_inputs(field_embeddings, w0, b0, w1, b1, w2, b2)
    res = bass_utils.run_bass_kernel_spmd(
        nc, in_maps, core_ids=list(range(NCORES)), trace=TRACE
    )
    _LAST_RESULTS = res
    outs = []
    for c in range(NCORES):
        o = res.results[c]["out"]                  # (4, 128, B_LOC) fp32
        full = o.reshape(512, B_LOC)               # [L0;L1;L2a;L2b]
        outs.append(full.T)                        # (B_LOC, 512)
    return np.ascontiguousarray(np.concatenate(outs, axis=0), dtype=np.float32)
